# revision 2
# baseline (speedup 1.0000x reference)
"""Trainium2 Bass kernel v2 for nn_DeepFM_55439437857626.

Key insight: output = softmax over 2 classes of (fm_linear + second_order
+ dnn_out). The FM second-order term is added to BOTH logits, so it cancels
in the softmax -- drop it entirely. Only the logit DIFFERENCE matters:
  d = (fm1-fm0) + (h3_1-h3_0);  p1 = sigmoid(d + cb), p0 = sigmoid(-d - cb)

Structure (8 cores, data-parallel over batch, 2048 rows/core):
  * user table: indirect gather [1M, 66] bf16 rows = [emb(64)|fm_diff|pad]
  * small-vocab features (hour/gender/age/attr, 74 rows padded to 128):
    NO gather -- host builds one-hot matrices (plain [128V, 2048] bf16 and
    scale-weighted variant), device matmuls against small tables:
      - feature-major x tiles for the DNN directly (tanh + scale baked in)
      - fm_diff sums via N=1 matmul
  * vis = visu_w @ visual^T in fp8 DoubleRow (visual pre-transposed, x4)
  * DNN h1/h2/h3 in fp8 DoubleRow (weights x32 host side, x features x64,
    rescaled in the PSUM->SBUF activation)
"""

import sys

sys.path.insert(0, "/opt/trn_rl_repo")

import numpy as np

import concourse.bacc as bacc
import concourse.bass as bass
import concourse.tile as tile
from concourse import mybir
from concourse.bass_utils import run_bass_kernel_spmd
from concourse.masks import make_identity

# ---- problem constants ----
NUM_USERS = 1_000_000
N_HOUR, N_GENDER, N_AGE, N_ATTR = 24, 2, 8, 40
K = 64
VIS = 2048
HID = 512
B = 16384
NCORES = 8
BC = B // NCORES          # 2048
P = 128
NT = BC // P              # 16
GT = 4
NG = NT // GT             # 4
V = 128                   # padded small-vocab size (74 used)
UC = 66                   # user row: emb(64) | fm_diff(1) | pad(1)

# vocab row offsets within V
OH_HOUR = 0
OH_GENDER = 24
OH_AGE = 26
OH_ATTR = 34

XS = 32.0       # x-feature fp8 scale
HS = 64.0       # hidden-activation fp8 scale
WSCALE = 32.0   # weight fp8 scale
VSCALE = 4.0    # visual fp8 scale

F32 = mybir.dt.float32
BF16 = mybir.dt.bfloat16
FP8 = mybir.dt.float8e4
I32 = mybir.dt.int32

AF = mybir.ActivationFunctionType
ALU = mybir.AluOpType
DR = mybir.MatmulPerfMode.DoubleRow


def build_nc():
    nc = bacc.Bacc(trn_type="TRN2")

    utab = nc.dram_tensor("utab", [NUM_USERS, UC], BF16, kind="ExternalInput")
    cols = nc.dram_tensor("cols", [P, NT], I32, kind="ExternalInput")
    oh1 = nc.dram_tensor("oh1", [V, BC], BF16, kind="ExternalInput")
    ohs = nc.dram_tensor("ohs", [V, BC], BF16, kind="ExternalInput")
    fmtab = nc.dram_tensor("fmtab", [V, 256], BF16, kind="ExternalInput")
    fmd = nc.dram_tensor("fmd", [V, 1], BF16, kind="ExternalInput")
    vtp = nc.dram_tensor("vtp", [NG, P, 16 * 512], FP8, kind="ExternalInput")
    w1t = nc.dram_tensor("w1t", [P, 3, HID], FP8, kind="ExternalInput")
    w2t = nc.dram_tensor("w2t", [P, 4, HID], FP8, kind="ExternalInput")
    w3dt = nc.dram_tensor("w3dt", [P, 4, 1], FP8, kind="ExternalInput")
    vwt = nc.dram_tensor("vwt", [P, 16, K], FP8, kind="ExternalInput")
    b1 = nc.dram_tensor("b1", [P, 4], F32, kind="ExternalInput")
    b2 = nc.dram_tensor("b2", [P, 4], F32, kind="ExternalInput")
    b2r = nc.dram_tensor("b2r", [1, HID], BF16, kind="ExternalInput")
    b1r = nc.dram_tensor("b1r", [1, HID], BF16, kind="ExternalInput")
    vb = nc.dram_tensor("vb", [K, 1], F32, kind="ExternalInput")
    out = nc.dram_tensor("out", [P, NT, 2], F32, kind="ExternalOutput")

    with tile.TileContext(nc) as tc:
        with (
            tc.tile_pool(name="singles", bufs=1) as singles,
            tc.tile_pool(name="ug", bufs=2) as ugpool,
            tc.tile_pool(name="xg", bufs=2) as xgpool,
            tc.tile_pool(name="hs", bufs=2) as hpool,
            tc.tile_pool(name="vload", bufs=2) as vpool,
            tc.tile_pool(name="small", bufs=3) as spool,
            tc.tile_pool(name="ps_mm", bufs=2, space="PSUM") as ps_mm,
            tc.tile_pool(name="ps_vis", bufs=1, space="PSUM") as ps_vis,
            tc.tile_pool(name="ps_fm", bufs=1, space="PSUM") as ps_fm,
            tc.tile_pool(name="ps_u", bufs=1, space="PSUM") as ps_u,
            tc.tile_pool(name="ps_d", bufs=2, space="PSUM") as ps_d,
        ):
            # ---- constants / weights, loaded once ----
            ident = singles.tile([P, P], BF16)
            make_identity(nc, ident[:])
            cols_s = singles.tile([P, NT], I32)
            nc.sync.dma_start(out=cols_s[:], in_=cols[:, :])
            fmtab_s = singles.tile([V, 256], BF16)
            nc.sync.dma_start(out=fmtab_s[:], in_=fmtab[:, :])
            fmd_s = singles.tile([V, 1], BF16)
            nc.sync.dma_start(out=fmd_s[:], in_=fmd[:, :])
            ohs_s = singles.tile([V, BC], BF16)
            nc.scalar.dma_start(out=ohs_s[:], in_=ohs[:, :])
            oh1_s = singles.tile([V, BC], BF16)
            nc.sync.dma_start(out=oh1_s[:], in_=oh1[:, :])
            vwt_s = singles.tile([P, 16, K], FP8)
            nc.sync.dma_start(out=vwt_s[:], in_=vwt[:, :, :])
            w1t_s = singles.tile([P, 3, HID], FP8)
            nc.sync.dma_start(out=w1t_s[:], in_=w1t[:, :, :])
            w2t_s = singles.tile([P, 4, HID], FP8)
            nc.sync.dma_start(out=w2t_s[:], in_=w2t[:, :, :])
            w3dt_s = singles.tile([P, 4, 1], FP8)
            nc.sync.dma_start(out=w3dt_s[:], in_=w3dt[:, :, :])
            b1_s = singles.tile([P, 4], F32)
            nc.sync.dma_start(out=b1_s[:], in_=b1[:, :])
            b2_s = singles.tile([P, 4], F32)
            nc.sync.dma_start(out=b2_s[:], in_=b2[:, :])
            vb_s = singles.tile([K, 1], F32)
            nc.sync.dma_start(out=vb_s[:], in_=vb[:, :])
            outbuf = singles.tile([P, NT, 2], F32)
            ones_s = singles.tile([1, HID], BF16)
            nc.vector.memset(ones_s[:], 1.0)
            b2r_s = singles.tile([1, HID], BF16)
            nc.sync.dma_start(out=b2r_s[:], in_=b2r[:, :])
            b1r_s = singles.tile([1, HID], BF16)
            nc.sync.dma_start(out=b1r_s[:], in_=b1r[:, :])
            warm = singles.tile([1, 1], F32)
            nc.scalar.activation(out=warm[:], in_=ones_s[0:1, 0:1],
                                 func=AF.Sigmoid)

            for g in range(NG):
                g4 = g * GT
                gs = slice(g4 * P, (g4 + GT) * P)  # group batch cols

                # ---------- user gathers (batch-on-partition) ----------
                ug = ugpool.tile([P, GT, UC], BF16, tag="ug")
                for t in range(GT):
                    nc.gpsimd.indirect_dma_start(
                        out=ug[:, t, :], out_offset=None, in_=utab[:, :],
                        in_offset=bass.IndirectOffsetOnAxis(
                            ap=cols_s[:, g4 + t : g4 + t + 1], axis=0),
                    )

                # ---------- visual slab (fp8, feature-major) ----------
                vslab = vpool.tile([P, 16, 512], FP8, tag="vslab")
                nc.sync.dma_start(
                    out=vslab[:, 0:8, :], in_=vtp[g, :, 0:4096])
                nc.gpsimd.dma_start(
                    out=vslab[:, 8:16, :], in_=vtp[g, :, 4096:8192])

                # ---------- x tiles (feature-major, fp8, x64) ----------
                xg = xgpool.tile([P, 3, HID], FP8, tag="xg")
                dps = ps_d.tile([P, 2, GT], F32, tag="dps")
                # [tg | tat] and [ta | hour] feature-major blocks, whole group
                fmps = ps_fm.tile([P, 2, HID], F32, tag="fmps")
                nc.tensor.matmul(
                    out=fmps[:, 0, :], lhsT=fmtab_s[:, 0:128],
                    rhs=ohs_s[:, gs], start=True, stop=True)
                nc.tensor.matmul(
                    out=fmps[0:64, 1, :], lhsT=fmtab_s[:, 128:192],
                    rhs=ohs_s[:, gs], start=True, stop=True)
                nc.tensor.matmul(
                    out=fmps[64:128, 1, :], lhsT=fmtab_s[:, 192:256],
                    rhs=oh1_s[:, gs], start=True, stop=True)
                nc.vector.tensor_copy(out=xg[:, 0:2, :], in_=fmps[:])
                # fm linear diff via N=1 matmul on the plain one-hot
                for t in range(GT):
                    cs = slice((g4 + t) * P, (g4 + t + 1) * P)
                    nc.tensor.matmul(
                        out=dps[:, 0, t : t + 1], lhsT=oh1_s[:, cs],
                        rhs=fmd_s[:, :], start=True, stop=True)
                # e = fm_d + user fm-diff (early, off the tail path)
                e_d = spool.tile([P, GT], F32, tag="e_d")
                nc.vector.tensor_tensor(
                    out=e_d[:], in0=dps[:, 0, :], in1=ug[:, :, 64],
                    op=ALU.add)

                # user^T batched (scale to xXS in the copy)
                psU = ps_u.tile([K, GT, P], BF16, tag="psU")
                for t in range(GT):
                    nc.tensor.transpose(
                        out=psU[:, t, :], in_=ug[:, t, 0:K],
                        identity=ident[:])
                nc.vector.tensor_scalar(
                    out=xg[0:K, 2, :], in0=psU[:], scalar1=XS,
                    scalar2=None, op0=ALU.mult)

                # ---------- vis = visu_w @ visual^T (fp8 DoubleRow) ----
                vis_ps = ps_vis.tile([K, HID], F32, tag="vis")
                for k in range(0, 16, 2):
                    nc.tensor.matmul(
                        out=vis_ps[:],
                        lhsT=vwt_s[:, k : k + 2, :],
                        rhs=vslab[:, k : k + 2, :],
                        start=(k == 0), stop=(k == 14),
                        perf_mode=DR)
                # xg2[64:128] = vis_ps * (FSCALE/(WSCALE*VSCALE)) + FSCALE*vb
                nc.vector.tensor_scalar(
                    out=xg[K:P, 2, :], in0=vis_ps[:],
                    scalar1=XS / (WSCALE * VSCALE),
                    scalar2=vb_s[:, 0:1], op0=ALU.mult, op1=ALU.add)

                # ---------- h1 = relu(w1 @ x + b1), fp8 x64 ----------
                h1t = hpool.tile([P, 4, HID], FP8, tag="h1t")
                for m in range(4):
                    ms = slice(m * P, (m + 1) * P)
                    on_act = m < 2
                    mm = ps_mm.tile([P, HID], F32, tag="mm")
                    nc.tensor.matmul(
                        out=mm[:], lhsT=w1t_s[:, 0:2, ms], rhs=xg[:, 0:2, :],
                        start=True, stop=False, perf_mode=DR)
                    nc.tensor.matmul(
                        out=mm[:], lhsT=w1t_s[:, 2, ms], rhs=xg[:, 2, :],
                        start=False, stop=on_act)
                    if on_act:
                        nc.scalar.activation(
                            out=h1t[:, m, :], in_=mm[:], func=AF.Relu,
                            bias=b1_s[:, m : m + 1], scale=HS / (WSCALE * XS))
                    else:
                        nc.tensor.matmul(
                            out=mm[:], lhsT=b1r_s[:, ms], rhs=ones_s[:, :],
                            start=False, stop=True)
                        nc.vector.tensor_scalar(
                            out=h1t[:, m, :], in0=mm[:],
                            scalar1=HS / (WSCALE * XS), scalar2=0.0,
                            op0=ALU.mult, op1=ALU.max)
                # ---------- h2 = relu(w2 @ h1 + b2), fp8 x64 ----------
                h2t = hpool.tile([P, 4, HID], FP8, tag="h2t")
                for m in range(4):
                    ms = slice(m * P, (m + 1) * P)
                    on_act = m < 2
                    mm = ps_mm.tile([P, HID], F32, tag="mm")
                    for kk in range(0, 4, 2):
                        nc.tensor.matmul(
                            out=mm[:], lhsT=w2t_s[:, kk : kk + 2, ms],
                            rhs=h1t[:, kk : kk + 2, :],
                            start=(kk == 0), stop=(kk == 2 and on_act),
                            perf_mode=DR)
                    if on_act:
                        nc.scalar.activation(
                            out=h2t[:, m, :], in_=mm[:], func=AF.Relu,
                            bias=b2_s[:, m : m + 1], scale=HS / (WSCALE * HS))
                    else:
                        nc.tensor.matmul(
                            out=mm[:], lhsT=b2r_s[:, ms], rhs=ones_s[:, :],
                            start=False, stop=True)
                        nc.vector.tensor_scalar(
                            out=h2t[:, m, :], in0=mm[:],
                            scalar1=1.0 / WSCALE, scalar2=0.0,
                            op0=ALU.mult, op1=ALU.max)

                # ---------- h3 diff + fm combine + sigmoid ----------
                for t in range(GT):
                    ls = slice(t * P, (t + 1) * P)
                    for kk in range(0, 4, 2):
                        nc.tensor.matmul(
                            out=dps[:, 1, t : t + 1],
                            lhsT=h2t[:, kk : kk + 2, ls],
                            rhs=w3dt_s[:, kk : kk + 2, :],
                            start=(kk == 0), stop=(kk == 2), perf_mode=DR)
                d = spool.tile([P, GT], F32, tag="d")
                nc.vector.tensor_scalar(
                    out=d[:], in0=dps[:, 1, :],
                    scalar1=1.0 / (WSCALE * HS),
                    scalar2=None, op0=ALU.mult)
                nc.vector.tensor_tensor(
                    out=d[:], in0=d[:], in1=e_d[:], op=ALU.add)
                nc.scalar.activation(
                    out=outbuf[:, g4 : g4 + GT, 1], in_=d[:], func=AF.Sigmoid)
                nc.scalar.activation(
                    out=outbuf[:, g4 : g4 + GT, 0], in_=d[:], func=AF.Sigmoid,
                    scale=-1.0)
                nc.sync.dma_start(
                    out=out[:, g4 : g4 + GT, :],
                    in_=outbuf[:, g4 : g4 + GT, :])

    nc.compile()
    return nc


def prep_inputs(inputs):
    """Host-side layout prep."""
    f32 = np.float32
    bf16 = mybir.dt.np(BF16)
    fp8 = mybir.dt.np(FP8)

    user_emb = np.asarray(inputs["user_emb"], f32)
    hour_emb = np.asarray(inputs["hour_emb"], f32)
    gender_emb = np.asarray(inputs["gender_emb"], f32)
    age_emb = np.asarray(inputs["age_emb"], f32)
    attr_emb = np.asarray(inputs["attr_emb"], f32)
    fm_w = np.asarray(inputs["fm_w"], f32)
    fm_wd = fm_w[1] - fm_w[0]  # [VOCAB]

    # user table: [emb(64) | fm_diff + const-bias | pad]
    fm_b = np.asarray(inputs["fm_b"], f32)
    b3 = np.asarray(inputs["b3"], f32)
    cb = (fm_b[1] - fm_b[0]) + (b3[1] - b3[0])
    utab = np.zeros((NUM_USERS, UC), bf16)
    utab[:, 0:K] = user_emb.astype(bf16)
    utab[:, 64] = (fm_wd[:NUM_USERS] + cb).astype(bf16)

    OFF_H = NUM_USERS
    OFF_G = OFF_H + N_HOUR
    OFF_AGE = OFF_G + N_GENDER
    OFF_ATTR = OFF_AGE + N_AGE

    # fm-major feature table (x64, tanh baked in): cols [tg|tat|ta|hour]
    fmtab = np.zeros((V, 256), f32)
    fmtab[OH_GENDER : OH_GENDER + N_GENDER, 0:64] = XS * np.tanh(gender_emb)
    fmtab[OH_ATTR : OH_ATTR + N_ATTR, 64:128] = XS * np.tanh(attr_emb)
    fmtab[OH_AGE : OH_AGE + N_AGE, 128:192] = XS * np.tanh(age_emb)
    fmtab[OH_HOUR : OH_HOUR + N_HOUR, 192:256] = XS * hour_emb

    # fm linear-diff column per vocab row
    fmd_t = np.zeros((V, 1), f32)
    fmd_t[OH_HOUR : OH_HOUR + N_HOUR, 0] = fm_wd[OFF_H : OFF_H + N_HOUR]
    fmd_t[OH_GENDER : OH_GENDER + N_GENDER, 0] = fm_wd[OFF_G : OFF_G + N_GENDER]
    fmd_t[OH_AGE : OH_AGE + N_AGE, 0] = fm_wd[OFF_AGE : OFF_AGE + N_AGE]
    fmd_t[OH_ATTR : OH_ATTR + N_ATTR, 0] = fm_wd[OFF_ATTR : OFF_ATTR + N_ATTR]

    user_id = np.asarray(inputs["user_id"]).astype(np.int64)
    hour = np.asarray(inputs["hour"]).astype(np.int64)
    gender = np.asarray(inputs["gender"]).astype(np.int64)
    age = np.asarray(inputs["age"]).astype(np.int64)
    attribute = np.asarray(inputs["attribute"]).astype(np.int64)
    scale = np.asarray(inputs["scale"], f32).reshape(B)
    visual = np.asarray(inputs["visual"], f32)

    # one-hots over the packed small vocab, per batch row
    rows = np.empty((4, B), np.int64)
    rows[0] = OH_HOUR + hour
    rows[1] = OH_GENDER + gender
    rows[2] = OH_AGE + age
    rows[3] = OH_ATTR + attribute
    bidx = np.arange(B)
    oh1_full = np.zeros((V, B), f32)
    ohs_full = np.zeros((V, B), f32)
    for r in range(4):
        oh1_full[rows[r], bidx] = 1.0
        ohs_full[rows[r], bidx] = scale

    # DNN weights; x feature order: [tg,tat | ta,hour | user,vis]
    w1 = np.asarray(inputs["w1"], f32)
    w2 = np.asarray(inputs["w2"], f32)
    w3 = np.asarray(inputs["w3"], f32)
    visu_w = np.asarray(inputs["visu_w"], f32)
    w1T = w1.T  # [384, 512]; orig rows: user 0:64, hr 64:128, tg 128:192,
    #             tat 192:256, ta 256:320, vis 320:384
    w1p = np.concatenate(
        [w1T[128:256], w1T[256:320], w1T[64:128], w1T[0:64], w1T[320:384]],
        axis=0)  # [384, 512] new order
    w1t = np.ascontiguousarray(
        (WSCALE * w1p).reshape(3, P, HID).transpose(1, 0, 2)).astype(fp8)
    w2t = np.ascontiguousarray(
        (WSCALE * w2.T).reshape(4, P, HID).transpose(1, 0, 2)).astype(fp8)
    w3d = (WSCALE * (w3[1] - w3[0])).reshape(4, P, 1)
    w3dt = np.ascontiguousarray(w3d.transpose(1, 0, 2)).astype(fp8)
    vwt = np.ascontiguousarray(
        (WSCALE * visu_w.T).reshape(16, P, K).transpose(1, 0, 2)).astype(fp8)
    b1 = np.ascontiguousarray(
        (HS * np.asarray(inputs["b1"], f32)).reshape(4, P).T)
    b2 = np.ascontiguousarray(
        (HS * np.asarray(inputs["b2"], f32)).reshape(4, P).T)
    b2r = (WSCALE * HS * np.asarray(inputs["b2"], f32)).reshape(1, HID).astype(bf16)
    b1r = (WSCALE * XS * np.asarray(inputs["b1"], f32)).reshape(1, HID).astype(bf16)
    vb = (XS * np.asarray(inputs["visu_b"], f32)).reshape(K, 1)

    shared = dict(utab=utab, fmtab=fmtab.astype(bf16), fmd=fmd_t.astype(bf16),
                  w1t=w1t, w2t=w2t, w3dt=w3dt, vwt=vwt, b1=b1, b2=b2,
                  b2r=b2r, b1r=b1r, vb=vb)
    in_maps = []
    for c in range(NCORES):
        s = slice(c * BC, (c + 1) * BC)
        m = dict(shared)
        m["cols"] = np.ascontiguousarray(
            user_id[s].reshape(NT, P).T).astype(np.int32)
        m["oh1"] = np.ascontiguousarray(oh1_full[:, s]).astype(bf16)
        m["ohs"] = np.ascontiguousarray(ohs_full[:, s]).astype(bf16)
        v = np.clip(VSCALE * visual[s].T, -240, 240).astype(fp8)  # [2048f, 2048b]
        m["vtp"] = np.ascontiguousarray(
            v.reshape(16, P, NG, 512).transpose(2, 1, 0, 3).reshape(
                NG, P, 16 * 512))
        in_maps.append(m)
    return in_maps


def unpack_out(res):
    outs = []
    for c in range(NCORES):
        op = res.results[c]["out"]
        outs.append(np.ascontiguousarray(op.transpose(1, 0, 2).reshape(BC, 2)))
    return np.concatenate(outs, axis=0)


_NC_CACHE = None
LAST_RESULTS = None


def kernel(**inputs) -> np.ndarray:
    global _NC_CACHE, LAST_RESULTS
    if _NC_CACHE is None:
        _NC_CACHE = build_nc()
    nc = _NC_CACHE
    in_maps = prep_inputs(inputs)
    res = run_bass_kernel_spmd(nc, in_maps, core_ids=list(range(NCORES)))
    LAST_RESULTS = res
    return unpack_out(res)
